# revision 35
# baseline (speedup 1.0000x reference)
"""AdaptiveAttention Trainium2 kernel.

reference:
  q = tanh(query @ Wq.T + bq); k = tanh(key @ Wk.T + bk)
  dot = q @ k.T ; qn,kn row norms
  cos = dot / max(qn*kn, eps)
  euc = -sqrt(max(qn^2+kn^2-2dot, 0))
  w = softmax(scoring_weights); scores = w0*dot + w1*cos + w2*euc
  attn = softmax(scores, -1); context = attn @ v
  returns (context, attn)

Sharding: 8 cores = (batch b = c//2) x (query half h = c%2). Fully
data-parallel, no collectives. Each core: q rows [h*1024,(h+1)*1024) of
batch b, full K/V of batch b.

Device pipeline per core:
  phase 1: bf16 tanh transforms (tanh output stored as float32r for
           full-rate tf32-like matmuls), row norms via squares (gpsimd)
           + ones-matmul, rsqrt via Ln/Exp, DRAM-bounce row<->column
           reshapes and partition broadcasts.
  phase 2 (software-pipelined across query tiles, 3 stages): per
           128-query tile: S=q@k^T into PSUM (two halves so the next
           tile's matmuls start early), v=qn2+kn2-2S fused as
           scalar_tensor_tensor + qn2 via the Ln bias, e'=w2*sqrt(v)
           via Ln/Exp (one activation-table set), p=S*(w0+w1*rq*rk)
           in one affine_mul_reduce, scores=p-e' with the row-sum
           accumulated in the same op (softmax shifts by the row MEAN
           - spread is ~38 < 88 so exp can't overflow, and any shift
           cancels), x=exp(scores-mean) in bf16 with the row sum from
           the activation's accum_out, attn=x/sum on gpsimd, xT via
           the DMA xbar transpose, ctx=xT.T@v (bf16), row-scaled by
           1/sum on the scalar engine.
  Activation tables are pinned (exp/ln only in natural_log_exp_and_others,
  tanh only in tanh_and_derivative) to avoid table-load thrash.
  gpsimd (SWDGE) queues are used only for output DMAs - input DMAs on
  that path have broken completion ordering vs compute on HW.
"""

import sys

if "/opt/trn_rl_repo" not in sys.path:
    sys.path.insert(0, "/opt/trn_rl_repo")

import numpy as np

B, LQ, LK, D = 4, 2048, 2048, 512
NCORES = 8
QSH = LQ // 2          # query rows per core
NT = QSH // 128        # 8 query tiles per core
KB = LK // 512         # 4 k blocks of 512
DC = D // 128          # 4 contraction chunks
EC = D // 128          # 4 output-feature chunks

_GRAPH_CACHE = {}
_TABLES_PATCHED = False


def _patch_act_tables():
    """Restrict exp/ln to natural_log_exp_and_others and tanh to
    tanh_and_derivative so the compiler's table-load pass can't alternate
    between sets (each ACT_TABLE_LOAD costs ~2.7us). Set ids must stay
    stable, so we keep the dict keys/order and only prune membership."""
    global _TABLES_PATCHED
    if _TABLES_PATCHED:
        return
    import concourse.bacc as bacc_mod

    orig = bacc_mod.get_activation_tables

    def patched(arch):
        out = {}
        for name, funcs in orig(arch).items():
            fs = set(funcs)
            if name != "natural_log_exp_and_others":
                fs = {f for f in fs if f.name not in ("Exp", "Ln")}
            if name != "tanh_and_derivative":
                fs = {f for f in fs if f.name != "Tanh"}
            out[name] = fs
        return out

    bacc_mod.get_activation_tables = patched
    _TABLES_PATCHED = True


def _build_graph(w0, w1, w2):
    import concourse.bass as bass
    import concourse.tile as tile
    from concourse import bacc, mybir

    _patch_act_tables()

    dt = mybir.dt
    AF = mybir.ActivationFunctionType
    ALU = mybir.AluOpType

    lnw2 = float(np.log(w2))

    nc = bacc.Bacc(None, target_bir_lowering=False)

    qT_d = nc.dram_tensor("qT", [D, QSH], dt.float32r, kind="ExternalInput")
    kT_d = nc.dram_tensor("kT", [D, LK], dt.float32r, kind="ExternalInput")
    v_d = nc.dram_tensor("v", [LK, D], dt.bfloat16, kind="ExternalInput")
    wqT_d = nc.dram_tensor("wqT", [D, D], dt.float32r, kind="ExternalInput")
    wkT_d = nc.dram_tensor("wkT", [D, D], dt.float32r, kind="ExternalInput")
    bq_d = nc.dram_tensor("bq", [128, EC], dt.float32, kind="ExternalInput")
    bk_d = nc.dram_tensor("bk", [128, EC], dt.float32, kind="ExternalInput")
    ones_d = nc.dram_tensor("ones", [128, 1], dt.bfloat16, kind="ExternalInput")
    ctx_d = nc.dram_tensor("ctx", [QSH, D], dt.float32, kind="ExternalOutput")
    attn_d = nc.dram_tensor("attn", [QSH, LK], dt.float32, kind="ExternalOutput")

    with tile.TileContext(nc) as tc:
        with tc.tile_pool(name="static", bufs=1) as st, \
             tc.tile_pool(name="strow", bufs=1) as strow, \
             tc.tile_pool(name="dram", bufs=1, space="DRAM") as drp:
            # persistent SBUF
            kTt = [st.tile([128, LK], dt.float32r, name=f"kTt{i}", tag=f"kTt{i}") for i in range(EC)]
            qTt = [st.tile([128, QSH], dt.float32r, name=f"qTt{i}", tag=f"qTt{i}") for i in range(EC)]
            vb = [st.tile([128, D], dt.bfloat16, name=f"vb{j}", tag=f"vb{j}") for j in range(LK // 128)]
            kn2b = st.tile([128, LK], dt.float32, name="kn2b", tag="kn2b")
            rkb = st.tile([128, LK], dt.float32, name="rkb", tag="rkb")
            qn2c = strow.tile([128, NT], dt.float32, name="qn2c", tag="qn2c")
            rqwc = strow.tile([128, NT], dt.float32, name="rqwc", tag="rqwc")
            ones_sb = strow.tile([128, 1], dt.bfloat16, name="ones", tag="ones")
            bq_sb = strow.tile([128, EC], dt.float32, name="bqs", tag="bqs")
            bk_sb = strow.tile([128, EC], dt.float32, name="bks", tag="bks")
            lnw2_sb = strow.tile([128, 1], dt.float32, name="lnw2c", tag="lnw2c")
            nc.vector.memset(lnw2_sb[:], lnw2)
            nc.sync.dma_start(ones_sb[:], ones_d[:])
            nc.sync.dma_start(bq_sb[:], bq_d[:])
            nc.sync.dma_start(bk_sb[:], bk_d[:])

            # DRAM scratch for row<->col moves
            kn2_dram = drp.tile([1, LK], dt.float32, name="kn2d")
            qn2_dram = drp.tile([1, QSH], dt.float32, name="qn2d")

            # ---------------- phase 1: transforms + norms ----------------
            with tc.tile_pool(name="raw", bufs=1) as raw, \
                 tc.tile_pool(name="sq", bufs=1) as sqp, \
                 tc.tile_pool(name="rows", bufs=2) as rows, \
                 tc.tile_pool(name="p1ps", bufs=3, space="PSUM") as p1ps:
                kraw = [raw.tile([128, LK], dt.float32r, name=f"kraw{i}", tag=f"kraw{i}") for i in range(DC)]
                qraw = [raw.tile([128, QSH], dt.float32r, name=f"qraw{i}", tag=f"qraw{i}") for i in range(DC)]
                wq_sb = [raw.tile([128, D], dt.float32r, name=f"wq{i}", tag=f"wq{i}") for i in range(DC)]
                wk_sb = [raw.tile([128, D], dt.float32r, name=f"wk{i}", tag=f"wk{i}") for i in range(DC)]
                # weights first, then k/q raw column-sliced so transform work
                # unblocks after the first slice of each chunk lands
                for i in range(DC):
                    nc.sync.dma_start(wk_sb[i][:], wkT_d[i * 128:(i + 1) * 128, :])
                    nc.scalar.dma_start(wq_sb[i][:], wqT_d[i * 128:(i + 1) * 128, :])
                for js in range(KB):
                    for i in range(DC):
                        nc.sync.dma_start(kraw[i][:, js * 512:(js + 1) * 512],
                                          kT_d[i * 128:(i + 1) * 128, js * 512:(js + 1) * 512])
                for js in range(QSH // 512):
                    for i in range(DC):
                        nc.scalar.dma_start(qraw[i][:, js * 512:(js + 1) * 512],
                                          qT_d[i * 128:(i + 1) * 128, js * 512:(js + 1) * 512])

                # k transform: kTt[E][:, js] = tanh(sum_dc wkT[dc][:,E*128:] ^T @ kraw[dc][:, js] + bk[E])
                ksq = [sqp.tile([128, LK], dt.bfloat16, name=f"ksq{i}", tag=f"ksq{i}") for i in range(EC)]
                qsq = [sqp.tile([128, QSH], dt.bfloat16, name=f"qsq{i}", tag=f"qsq{i}") for i in range(EC)]
                kn2row = rows.tile([1, LK], dt.float32, name="kn2row", tag="kn2row")
                qn2row = rows.tile([1, QSH], dt.float32, name="qn2row", tag="qn2row")
                for js in range(KB):
                    sl = slice(js * 512, (js + 1) * 512)
                    for E in range(EC):
                        ps = p1ps.tile([128, 512], dt.float32, name="tps", tag="tps")
                        for dc in range(DC):
                            nc.tensor.matmul(
                                ps[:], wk_sb[dc][:, E * 128:(E + 1) * 128],
                                kraw[dc][:, sl],
                                start=(dc == 0), stop=(dc == DC - 1))
                        nc.scalar.activation(
                            out=kTt[E][:, sl], in_=ps[:],
                            func=AF.Tanh, bias=bk_sb[:, E:E + 1], scale=1.0)
                        nc.gpsimd.tensor_tensor(
                            out=ksq[E][:, sl], in0=kTt[E][:, sl].bitcast(dt.float32),
                            in1=kTt[E][:, sl].bitcast(dt.float32), op=ALU.mult)
                    # norm slice for this js as soon as its squares exist
                    nps = p1ps.tile([1, 512], dt.float32, name="nps", tag="tps")
                    for E in range(EC):
                        nc.tensor.matmul(nps[:], ones_sb[:], ksq[E][:, sl],
                                         start=(E == 0), stop=(E == EC - 1))
                    nc.vector.tensor_copy(out=kn2row[:, sl], in_=nps[:])

                # kn2 broadcast + column form (no ACT needed -> runs during q side)
                nc.scalar.dma_start(kn2_dram[:], kn2row[:])
                bc_src = bass.AP(tensor=kn2_dram.tensor, offset=kn2_dram[:].offset,
                                 ap=[[0, 128], [1, LK]])
                nc.sync.dma_start(kn2b[:], bc_src)
                kn2col = rows.tile([128, LK // 128], dt.float32, name="kn2col", tag="kn2col")
                kcol_src = bass.AP(tensor=kn2_dram.tensor, offset=kn2_dram[:].offset,
                                   ap=[[1, 128], [128, LK // 128]])
                nc.scalar.dma_start(kn2col[:], kcol_src)

                # q transform (+ squares + norm slices)
                for js in range(QSH // 512):
                    sl = slice(js * 512, (js + 1) * 512)
                    for E in range(EC):
                        ps = p1ps.tile([128, 512], dt.float32, name="tps", tag="tps")
                        for dc in range(DC):
                            nc.tensor.matmul(
                                ps[:], wq_sb[dc][:, E * 128:(E + 1) * 128],
                                qraw[dc][:, sl],
                                start=(dc == 0), stop=(dc == DC - 1))
                        nc.scalar.activation(
                            out=qTt[E][:, sl], in_=ps[:],
                            func=AF.Tanh, bias=bq_sb[:, E:E + 1], scale=1.0)
                        nc.gpsimd.tensor_tensor(
                            out=qsq[E][:, sl], in0=qTt[E][:, sl].bitcast(dt.float32),
                            in1=qTt[E][:, sl].bitcast(dt.float32), op=ALU.mult)
                    nps = p1ps.tile([1, 512], dt.float32, name="nps", tag="tps")
                    for E in range(EC):
                        nc.tensor.matmul(nps[:], ones_sb[:], qsq[E][:, sl],
                                         start=(E == 0), stop=(E == EC - 1))
                    nc.vector.tensor_copy(out=qn2row[:, sl], in_=nps[:])

                # qn2 row -> per-tile columns [128, NT] via DRAM bounce
                nc.scalar.dma_start(qn2_dram[:], qn2row[:])
                col_src = bass.AP(tensor=qn2_dram.tensor, offset=qn2_dram[:].offset,
                                  ap=[[1, 128], [128, NT]])
                nc.scalar.dma_start(qn2c[:], col_src)

                # rsqrts (single table switch to the ln/exp set)
                rkcol = rows.tile([128, LK // 128], dt.float32, name="rkcol", tag="rkcol")
                nc.scalar.activation(out=rkcol[:], in_=kn2col[:], func=AF.Ln)
                nc.scalar.activation(out=rkcol[:], in_=rkcol[:], func=AF.Exp, scale=-0.5)
                rk_dram = drp.tile([1, LK], dt.float32, name="rkd")
                rkd_dst = bass.AP(tensor=rk_dram.tensor, offset=rk_dram[:].offset,
                                  ap=[[1, 128], [128, LK // 128]])
                nc.scalar.dma_start(rkd_dst, rkcol[:])
                rkb_src = bass.AP(tensor=rk_dram.tensor, offset=rk_dram[:].offset,
                                  ap=[[0, 128], [1, LK]])
                nc.sync.dma_start(rkb[:], rkb_src)
                # rqw = w1 * rsqrt(qn2)
                nc.scalar.activation(out=rqwc[:], in_=qn2c[:], func=AF.Ln)
                nc.scalar.activation(out=rqwc[:], in_=rqwc[:], func=AF.Exp, scale=-0.5)
                nc.vector.tensor_scalar_mul(rqwc[:], rqwc[:], float(w1))

                # v loads straight into bf16 tiles (emitted last so they queue
                # behind everything the main loop needs first)
                for j in range(LK // 128):
                    nc.sync.dma_start(vb[j][:], v_d[j * 128:(j + 1) * 128, :])

            # ---------------- phase 2: attention ----------------
            with tc.tile_pool(name="sps", bufs=3, space="PSUM") as sps, \
                 tc.tile_pool(name="cps", bufs=2, space="PSUM") as cpsp, \
                 tc.tile_pool(name="ve", bufs=3) as vep, \
                 tc.tile_pool(name="pp", bufs=3) as ppp, \
                 tc.tile_pool(name="xp", bufs=3) as xpp, \
                 tc.tile_pool(name="xt", bufs=3) as xtp, \
                 tc.tile_pool(name="at", bufs=3) as atp, \
                 tc.tile_pool(name="cs", bufs=2) as csp, \
                 tc.tile_pool(name="cols", bufs=16) as cols:
                # software-pipelined by one tile: iteration t emits the head
                # of tile t (S matmuls + the two PSUM readers + ln/e') and the
                # tail of tile t-1 (scores, exp, attn, transpose, ctx) so the
                # serial v->ln->e'->scores->exp chain overlaps across tiles.
                live = {}
                live2 = {}
                for t in range(NT + 2):
                    if t < NT:
                        # S in two psum halves so the next tile's matmuls can
                        # start as soon as the first half's readers are done
                        ve = vep.tile([128, LK], dt.float32, name="ve", tag="ve")
                        p = ppp.tile([128, LK], dt.float32, name="p", tag="p")
                        junk = cols.tile([128, 1], dt.float32, name="junk", tag="junk")
                        for h in range(2):
                            hs = slice(h * 1024, (h + 1) * 1024)
                            Sh = sps.tile([128, LK // 2], dt.float32, name=f"S{h}", tag="S")
                            for kb in range(2):
                                for dc in range(DC):
                                    nc.tensor.matmul(
                                        Sh[:, kb * 512:(kb + 1) * 512],
                                        qTt[dc][:, t * 128:(t + 1) * 128],
                                        kTt[dc][:, (2 * h + kb) * 512:(2 * h + kb + 1) * 512],
                                        start=(dc == 0), stop=(dc == DC - 1))
                            # ve = kn2 - 2S
                            nc.vector.scalar_tensor_tensor(
                                out=ve[:, hs], in0=Sh[:], scalar=-2.0, in1=kn2b[:, hs],
                                op0=ALU.mult, op1=ALU.add)
                            # p = (rk*w1*rq + w0) * S  (frees this S half)
                            nc.vector.affine_mul_reduce(
                                out=p[:, hs], accum_out=junk[:], in0=rkb[:, hs], in1=Sh[:],
                                scale=rqwc[:, t:t + 1], bias=float(w0))
                        # ln(ve + qn2) ; then w2*sqrt(ve)
                        nc.scalar.activation(out=ve[:], in_=ve[:], func=AF.Ln,
                                             bias=qn2c[:, t:t + 1], scale=1.0)
                        nc.scalar.activation(out=ve[:], in_=ve[:], func=AF.Exp,
                                             bias=lnw2_sb[:, 0:1], scale=0.5)
                        live[t] = (ve, p)

                    if 1 <= t <= NT:
                        u = t - 1
                        ve, p = live.pop(u)
                        # scores = p - e' (in place into p), row sum for mean
                        ssum = cols.tile([128, 1], dt.float32, name="ssum", tag="ssum")
                        nc.vector.scalar_tensor_tensor(
                            out=p[:], in0=ve[:], scalar=-1.0, in1=p[:],
                            op0=ALU.mult, op1=ALU.add, accum_out=ssum[:])
                        negmean = cols.tile([128, 1], dt.float32, name="negmean", tag="negmean")
                        nc.scalar.activation(out=negmean[:], in_=ssum[:], func=AF.Copy,
                                             scale=-1.0 / LK)

                        # x = exp(scores - mean) (bf16), row sum
                        x = xpp.tile([128, LK], dt.bfloat16, name="x", tag="x")
                        sx = cols.tile([128, 1], dt.float32, name="sx", tag="sx")
                        nc.scalar.activation(out=x[:], in_=p[:], func=AF.Exp,
                                             bias=negmean[:, 0:1], scale=1.0,
                                             accum_out=sx[:])
                        rs = cols.tile([128, 1], dt.float32, name="rs", tag="rs")
                        nc.vector.reciprocal(rs[:], sx[:])

                        # xT[kk, c, i] = x[i, c*128+kk] via xbar transpose
                        # (emitted before the attn output so it isn't queued
                        # behind a 1MB DMA on the SP queue)
                        xT = xtp.tile([128, LK // 128, 128], dt.bfloat16, name="xT", tag="xT")
                        nc.sync.dma_start_transpose(xT[:], x[:])
                        live2[u] = (xT, rs)

                        # attn = x * (1/sum)  (gpsimd) -> DRAM
                        at = atp.tile([128, LK], dt.float32, name="at", tag="at")
                        nc.gpsimd.tensor_scalar(out=at[:], in0=x[:], scalar1=rs[:, 0:1],
                                                scalar2=None, op0=ALU.mult)
                        eng = nc.sync if u % 2 == 0 else nc.gpsimd
                        eng.dma_start(attn_d[u * 128:(u + 1) * 128, :], at[:])

                    if t >= 2:
                        u2 = t - 2
                        xT, rs = live2.pop(u2)
                        # ctx = x @ v (unnormalized), then scale rows by 1/sum
                        cps = cpsp.tile([128, D], dt.float32, name="cps", tag="cps")
                        for kc in range(LK // 128):
                            nc.tensor.matmul(cps[:], xT[:, kc, :], vb[kc][:],
                                             start=(kc == 0), stop=(kc == LK // 128 - 1))
                        csb = csp.tile([128, D], dt.float32, name="csb", tag="csb")
                        nc.scalar.activation(out=csb[:], in_=cps[:], func=AF.Copy,
                                             scale=rs[:, 0:1])
                        nc.gpsimd.dma_start(ctx_d[u2 * 128:(u2 + 1) * 128, :], csb[:])

    nc.compile()
    return nc


def _get_graph(w0, w1, w2):
    key = (round(float(w0), 9), round(float(w1), 9), round(float(w2), 9))
    if key not in _GRAPH_CACHE:
        _GRAPH_CACHE[key] = _build_graph(*key)
    return _GRAPH_CACHE[key]


def kernel(query, key, value, Wq, bq, Wk, bk, scoring_weights):
    import ml_dtypes
    from concourse.bass_utils import run_bass_kernel_spmd

    query = np.asarray(query, dtype=np.float32)
    key_ = np.asarray(key, dtype=np.float32)
    value = np.asarray(value, dtype=np.float32)
    Wq = np.asarray(Wq, dtype=np.float32)
    bq = np.asarray(bq, dtype=np.float32)
    Wk = np.asarray(Wk, dtype=np.float32)
    bk = np.asarray(bk, dtype=np.float32)
    sw = np.asarray(scoring_weights, dtype=np.float64)

    e = np.exp(sw - sw.max())
    w = (e / e.sum()).astype(np.float64)
    w0, w1, w2 = float(w[0]), float(w[1]), float(w[2])

    nc = _get_graph(w0, w1, w2)

    wqT = np.ascontiguousarray(Wq.T)
    wkT = np.ascontiguousarray(Wk.T)
    bq_c = np.ascontiguousarray(bq.reshape(EC, 128).T)
    bk_c = np.ascontiguousarray(bk.reshape(EC, 128).T)
    ones = np.ones((128, 1), dtype=ml_dtypes.bfloat16)

    bf16 = ml_dtypes.bfloat16
    in_maps = []
    for c in range(NCORES):
        b, h = c // 2, c % 2
        in_maps.append({
            "qT": np.ascontiguousarray(query[b, h * QSH:(h + 1) * QSH, :].T),
            "kT": np.ascontiguousarray(key_[b].T),
            "v": np.ascontiguousarray(value[b]).astype(bf16),
            "wqT": wqT, "wkT": wkT, "bq": bq_c, "bk": bk_c, "ones": ones,
        })

    res = run_bass_kernel_spmd(nc, in_maps, core_ids=list(range(NCORES)))

    context = np.empty((B, LQ, D), dtype=np.float32)
    attn = np.empty((B, LQ, LK), dtype=np.float32)
    for c in range(NCORES):
        b, h = c // 2, c % 2
        context[b, h * QSH:(h + 1) * QSH] = res.results[c]["ctx"]
        attn[b, h * QSH:(h + 1) * QSH] = res.results[c]["attn"]

    return context, attn


# revision 53
# speedup vs baseline: 1.1892x; 1.1892x over previous
"""AdaptiveAttention Trainium2 kernel.

reference:
  q = tanh(query @ Wq.T + bq); k = tanh(key @ Wk.T + bk)
  dot = q @ k.T ; qn,kn row norms
  cos = dot / max(qn*kn, eps)
  euc = -sqrt(max(qn^2+kn^2-2dot, 0))
  w = softmax(scoring_weights); scores = w0*dot + w1*cos + w2*euc
  attn = softmax(scores, -1); context = attn @ v
  returns (context, attn)

Sharding: 8 cores = (batch b = c//2) x (query half h = c%2). Fully
data-parallel, no collectives. Each core: q rows [h*1024,(h+1)*1024) of
batch b, full K/V of batch b.

Device pipeline per core:
  phase 1: bf16 tanh transforms (tanh output stored as float32r for
           full-rate tf32-like matmuls), row norms via squares (gpsimd)
           + ones-matmul, rsqrt via Ln/Exp, DRAM-bounce row<->column
           reshapes and partition broadcasts.
  phase 2 (software-pipelined across query tiles, 3 stages): per
           128-query tile: S=q@k^T into PSUM (two halves so the next
           tile's matmuls start early), v=qn2+kn2-2S fused as
           scalar_tensor_tensor + qn2 via the Ln bias, e'=w2*sqrt(v)
           via Ln/Exp (one activation-table set), p=S*(w0+w1*rq*rk)
           in one affine_mul_reduce, scores=p-e' with the row-sum
           accumulated in the same op (softmax shifts by the row MEAN
           - spread is ~38 < 88 so exp can't overflow, and any shift
           cancels), x=exp(scores-mean) in bf16 with the row sum from
           the activation's accum_out, attn=x/sum on gpsimd, xT via
           the DMA xbar transpose, ctx=xT.T@v (bf16), row-scaled by
           1/sum on the scalar engine.
  Activation tables are pinned (exp/ln only in natural_log_exp_and_others,
  tanh only in tanh_and_derivative) to avoid table-load thrash.
  gpsimd (SWDGE) queues are used only for output DMAs - input DMAs on
  that path have broken completion ordering vs compute on HW.
"""

import sys

if "/opt/trn_rl_repo" not in sys.path:
    sys.path.insert(0, "/opt/trn_rl_repo")

import numpy as np

B, LQ, LK, D = 4, 2048, 2048, 512
NCORES = 8
QSH = LQ // 2          # query rows per core
NT = QSH // 128        # 8 query tiles per core
KB = LK // 512         # 4 k blocks of 512
DC = D // 128          # 4 contraction chunks
EC = D // 128          # 4 output-feature chunks

_GRAPH_CACHE = {}
_TABLES_PATCHED = False


def _patch_act_tables():
    """Restrict exp/ln to natural_log_exp_and_others and tanh to
    tanh_and_derivative so the compiler's table-load pass can't alternate
    between sets (each ACT_TABLE_LOAD costs ~2.7us). Set ids must stay
    stable, so we keep the dict keys/order and only prune membership."""
    global _TABLES_PATCHED
    if _TABLES_PATCHED:
        return
    import concourse.bacc as bacc_mod

    orig = bacc_mod.get_activation_tables

    def patched(arch):
        out = {}
        for name, funcs in orig(arch).items():
            fs = set(funcs)
            if name != "natural_log_exp_and_others":
                fs = {f for f in fs if f.name not in ("Exp", "Ln")}
            if name != "tanh_and_derivative":
                fs = {f for f in fs if f.name != "Tanh"}
            out[name] = fs
        return out

    bacc_mod.get_activation_tables = patched
    _TABLES_PATCHED = True


def _build_graph(w0, w1, w2):
    import concourse.bass as bass
    import concourse.tile as tile
    from concourse import bacc, mybir
    from concourse.tile_rust import add_dep_helper

    _patch_act_tables()

    dt = mybir.dt
    AF = mybir.ActivationFunctionType
    ALU = mybir.AluOpType

    lnw2 = float(np.log(w2))

    nc = bacc.Bacc(None, target_bir_lowering=False)

    qT_d = nc.dram_tensor("qT", [D, QSH], dt.float32r, kind="ExternalInput")
    kT_d = nc.dram_tensor("kT", [D, LK], dt.float32r, kind="ExternalInput")
    v_d = nc.dram_tensor("v", [LK, D], dt.bfloat16, kind="ExternalInput")
    wqT_d = nc.dram_tensor("wqT", [D, D], dt.float32r, kind="ExternalInput")
    wkT_d = nc.dram_tensor("wkT", [D, D], dt.float32r, kind="ExternalInput")
    bq_d = nc.dram_tensor("bq", [128, EC], dt.float32, kind="ExternalInput")
    bk_d = nc.dram_tensor("bk", [128, EC], dt.float32, kind="ExternalInput")
    ones_d = nc.dram_tensor("ones", [128, 1], dt.bfloat16, kind="ExternalInput")
    ctx_d = nc.dram_tensor("ctx", [QSH, D], dt.float32, kind="ExternalOutput")
    attn_d = nc.dram_tensor("attn", [QSH, LK], dt.float32, kind="ExternalOutput")

    with tile.TileContext(nc) as tc:
        with tc.tile_pool(name="static", bufs=1) as st, \
             tc.tile_pool(name="strow", bufs=1) as strow, \
             tc.tile_pool(name="dram", bufs=1, space="DRAM") as drp:
            # persistent SBUF
            kTt = [st.tile([128, LK], dt.float32r, name=f"kTt{i}", tag=f"kTt{i}") for i in range(EC)]
            qTt = [st.tile([128, QSH], dt.float32r, name=f"qTt{i}", tag=f"qTt{i}") for i in range(EC)]
            vb = [st.tile([128, D], dt.bfloat16, name=f"vb{j}", tag=f"vb{j}") for j in range(LK // 128)]
            kn2b = st.tile([128, LK], dt.float32, name="kn2b", tag="kn2b")
            rkb = st.tile([128, LK], dt.float32, name="rkb", tag="rkb")
            qn2c = strow.tile([128, NT], dt.float32, name="qn2c", tag="qn2c")
            rqwc = strow.tile([128, NT], dt.float32, name="rqwc", tag="rqwc")
            ones_sb = strow.tile([128, 1], dt.bfloat16, name="ones", tag="ones")
            bq_sb = strow.tile([128, EC], dt.float32, name="bqs", tag="bqs")
            bk_sb = strow.tile([128, EC], dt.float32, name="bks", tag="bks")
            lnw2_sb = strow.tile([128, 1], dt.float32, name="lnw2c", tag="lnw2c")
            nc.vector.memset(lnw2_sb[:], lnw2)
            nc.sync.dma_start(ones_sb[:], ones_d[:])
            nc.sync.dma_start(bq_sb[:], bq_d[:])
            nc.sync.dma_start(bk_sb[:], bk_d[:])

            # DRAM scratch for row<->col moves
            kn2_dram = drp.tile([1, LK], dt.float32, name="kn2d")
            qn2_dram = drp.tile([1, QSH], dt.float32, name="qn2d")

            # ---------------- phase 1: transforms + norms ----------------
            with tc.tile_pool(name="raw", bufs=1) as raw, \
                 tc.tile_pool(name="sq", bufs=1) as sqp, \
                 tc.tile_pool(name="rows", bufs=2) as rows, \
                 tc.tile_pool(name="p1ps", bufs=3, space="PSUM") as p1ps:
                kraw = [raw.tile([128, LK], dt.float32r, name=f"kraw{i}", tag=f"kraw{i}") for i in range(DC)]
                qraw = [raw.tile([128, QSH], dt.float32r, name=f"qraw{i}", tag=f"qraw{i}") for i in range(DC)]
                wq_sb = [raw.tile([128, D], dt.float32r, name=f"wq{i}", tag=f"wq{i}") for i in range(DC)]
                wk_sb = [raw.tile([128, D], dt.float32r, name=f"wk{i}", tag=f"wk{i}") for i in range(DC)]
                # weights first, then k/q raw column-sliced so transform work
                # unblocks after the first slice of each chunk lands
                for i in range(DC):
                    nc.sync.dma_start(wk_sb[i][:], wkT_d[i * 128:(i + 1) * 128, :])
                    nc.scalar.dma_start(wq_sb[i][:], wqT_d[i * 128:(i + 1) * 128, :])
                for js in range(KB):
                    for i in range(DC):
                        nc.sync.dma_start(kraw[i][:, js * 512:(js + 1) * 512],
                                          kT_d[i * 128:(i + 1) * 128, js * 512:(js + 1) * 512])
                for js in range(QSH // 512):
                    for i in range(DC):
                        nc.scalar.dma_start(qraw[i][:, js * 512:(js + 1) * 512],
                                          qT_d[i * 128:(i + 1) * 128, js * 512:(js + 1) * 512])

                # k transform: kTt[E][:, js] = tanh(sum_dc wkT[dc][:,E*128:] ^T @ kraw[dc][:, js] + bk[E])
                ksq = [sqp.tile([128, LK], dt.bfloat16, name=f"ksq{i}", tag=f"ksq{i}") for i in range(EC)]
                qsq = [sqp.tile([128, QSH], dt.bfloat16, name=f"qsq{i}", tag=f"qsq{i}") for i in range(EC)]
                kn2row = rows.tile([1, LK], dt.float32, name="kn2row", tag="kn2row")
                qn2row = rows.tile([1, QSH], dt.float32, name="qn2row", tag="qn2row")
                for js in range(KB):
                    sl = slice(js * 512, (js + 1) * 512)
                    for E in range(EC):
                        ps = p1ps.tile([128, 512], dt.float32, name="tps", tag="tps")
                        for dc in range(DC):
                            nc.tensor.matmul(
                                ps[:], wk_sb[dc][:, E * 128:(E + 1) * 128],
                                kraw[dc][:, sl],
                                start=(dc == 0), stop=(dc == DC - 1))
                        nc.scalar.activation(
                            out=kTt[E][:, sl], in_=ps[:],
                            func=AF.Tanh, bias=bk_sb[:, E:E + 1], scale=1.0)
                        nc.gpsimd.tensor_tensor(
                            out=ksq[E][:, sl], in0=kTt[E][:, sl].bitcast(dt.float32),
                            in1=kTt[E][:, sl].bitcast(dt.float32), op=ALU.mult)
                    # norm slice for this js as soon as its squares exist
                    nps = p1ps.tile([1, 512], dt.float32, name="nps", tag="tps")
                    for E in range(EC):
                        nc.tensor.matmul(nps[:], ones_sb[:], ksq[E][:, sl],
                                         start=(E == 0), stop=(E == EC - 1))
                    nc.vector.tensor_copy(out=kn2row[:, sl], in_=nps[:])

                # kn2 broadcast + column form (no ACT needed -> runs during q side)
                nc.scalar.dma_start(kn2_dram[:], kn2row[:])
                bc_src = bass.AP(tensor=kn2_dram.tensor, offset=kn2_dram[:].offset,
                                 ap=[[0, 128], [1, LK]])
                nc.sync.dma_start(kn2b[:], bc_src)
                kn2col = rows.tile([128, LK // 128], dt.float32, name="kn2col", tag="kn2col")
                kcol_src = bass.AP(tensor=kn2_dram.tensor, offset=kn2_dram[:].offset,
                                   ap=[[1, 128], [128, LK // 128]])
                nc.scalar.dma_start(kn2col[:], kcol_src)

                # q transform (+ squares + norm slices)
                for js in range(QSH // 512):
                    sl = slice(js * 512, (js + 1) * 512)
                    for E in range(EC):
                        ps = p1ps.tile([128, 512], dt.float32, name="tps", tag="tps")
                        for dc in range(DC):
                            nc.tensor.matmul(
                                ps[:], wq_sb[dc][:, E * 128:(E + 1) * 128],
                                qraw[dc][:, sl],
                                start=(dc == 0), stop=(dc == DC - 1))
                        last_tanh = nc.scalar.activation(
                            out=qTt[E][:, sl], in_=ps[:],
                            func=AF.Tanh, bias=bq_sb[:, E:E + 1], scale=1.0)
                        nc.gpsimd.tensor_tensor(
                            out=qsq[E][:, sl], in0=qTt[E][:, sl].bitcast(dt.float32),
                            in1=qTt[E][:, sl].bitcast(dt.float32), op=ALU.mult)
                    nps = p1ps.tile([1, 512], dt.float32, name="nps", tag="tps")
                    for E in range(EC):
                        nc.tensor.matmul(nps[:], ones_sb[:], qsq[E][:, sl],
                                         start=(E == 0), stop=(E == EC - 1))
                    nc.vector.tensor_copy(out=qn2row[:, sl], in_=nps[:])

                # qn2 row -> per-tile columns [128, NT] via DRAM bounce
                nc.scalar.dma_start(qn2_dram[:], qn2row[:])
                col_src = bass.AP(tensor=qn2_dram.tensor, offset=qn2_dram[:].offset,
                                  ap=[[1, 128], [128, NT]])
                nc.scalar.dma_start(qn2c[:], col_src)

                # rsqrts (single table switch to the ln/exp set; the dep
                # edge keeps them after ALL tanh ops so the scheduler cannot
                # interleave them and thrash the activation tables)
                rkcol = rows.tile([128, LK // 128], dt.float32, name="rkcol", tag="rkcol")
                first_ln = nc.scalar.activation(out=rkcol[:], in_=kn2col[:], func=AF.Ln)
                add_dep_helper(first_ln.ins, last_tanh.ins,
                               reason="rsqrt after all tanh (act-table order)")
                nc.scalar.activation(out=rkcol[:], in_=rkcol[:], func=AF.Exp, scale=-0.5)
                rk_dram = drp.tile([1, LK], dt.float32, name="rkd")
                rkd_dst = bass.AP(tensor=rk_dram.tensor, offset=rk_dram[:].offset,
                                  ap=[[1, 128], [128, LK // 128]])
                nc.scalar.dma_start(rkd_dst, rkcol[:])
                rkb_src = bass.AP(tensor=rk_dram.tensor, offset=rk_dram[:].offset,
                                  ap=[[0, 128], [1, LK]])
                nc.sync.dma_start(rkb[:], rkb_src)
                # rqw = w1 * rsqrt(qn2)
                nc.scalar.activation(out=rqwc[:], in_=qn2c[:], func=AF.Ln)
                nc.scalar.activation(out=rqwc[:], in_=rqwc[:], func=AF.Exp, scale=-0.5)
                nc.vector.tensor_scalar_mul(rqwc[:], rqwc[:], float(w1))

                # v loads straight into bf16 tiles (emitted last so they queue
                # behind everything the main loop needs first)
                for j in range(LK // 128):
                    nc.sync.dma_start(vb[j][:], v_d[j * 128:(j + 1) * 128, :])

            # ---------------- phase 2: attention ----------------
            with tc.tile_pool(name="sps", bufs=3, space="PSUM") as sps, \
                 tc.tile_pool(name="cps", bufs=2, space="PSUM") as cpsp, \
                 tc.tile_pool(name="ve", bufs=3) as vep, \
                 tc.tile_pool(name="pp", bufs=3) as ppp, \
                 tc.tile_pool(name="xp", bufs=3) as xpp, \
                 tc.tile_pool(name="xt", bufs=3) as xtp, \
                 tc.tile_pool(name="at", bufs=3) as atp, \
                 tc.tile_pool(name="cs", bufs=2) as csp, \
                 tc.tile_pool(name="cols", bufs=16) as cols:
                # software-pipelined by one tile: iteration t emits the head
                # of tile t (S matmuls + the two PSUM readers + ln/e') and the
                # tail of tile t-1 (scores, exp, attn, transpose, ctx) so the
                # serial v->ln->e'->scores->exp chain overlaps across tiles.
                live = {}
                live2 = {}
                for t in range(NT + 2):
                    if t >= 2:
                        u2 = t - 2
                        xT, rs = live2.pop(u2)
                        # ctx = x @ v (unnormalized), then scale rows by 1/sum
                        cps = cpsp.tile([128, D], dt.float32, name="cps", tag="cps")
                        for kc in range(LK // 128):
                            nc.tensor.matmul(cps[:], xT[:, kc, :], vb[kc][:],
                                             start=(kc == 0), stop=(kc == LK // 128 - 1))
                        csb = csp.tile([128, D], dt.float32, name="csb", tag="csb")
                        nc.scalar.activation(out=csb[:, :D // 2], in_=cps[:, :D // 2],
                                             func=AF.Copy, scale=rs[:, 0:1])
                        nc.vector.tensor_scalar_mul(csb[:, D // 2:], cps[:, D // 2:],
                                                    rs[:, 0:1])
                        ceng = nc.gpsimd if u2 >= NT - 2 else nc.sync
                        ceng.dma_start(ctx_d[u2 * 128:(u2 + 1) * 128, :], csb[:])

                    if t < NT:
                        # S in two psum halves so the next tile's matmuls can
                        # start as soon as the first half's readers are done
                        ve = vep.tile([128, LK], dt.float32, name="ve", tag="ve")
                        p = ppp.tile([128, LK], dt.float32, name="p", tag="p")
                        psums = []
                        for h in range(2):
                            hs = slice(h * 1024, (h + 1) * 1024)
                            Sh = sps.tile([128, LK // 2], dt.float32, name=f"S{h}", tag="S")
                            for kb in range(2):
                                for dc in range(DC):
                                    nc.tensor.matmul(
                                        Sh[:, kb * 512:(kb + 1) * 512],
                                        qTt[dc][:, t * 128:(t + 1) * 128],
                                        kTt[dc][:, (2 * h + kb) * 512:(2 * h + kb + 1) * 512],
                                        start=(dc == 0), stop=(dc == DC - 1))
                            # ve = kn2 - 2S
                            nc.vector.scalar_tensor_tensor(
                                out=ve[:, hs], in0=Sh[:], scalar=-2.0, in1=kn2b[:, hs],
                                op0=ALU.mult, op1=ALU.add)
                            # p = (rk*w1*rq + w0) * S  (frees this S half);
                            # accum gives sum(p) for the softmax mean shift
                            ph = cols.tile([128, 1], dt.float32, name=f"ps{h}", tag=f"ps{h}")
                            nc.vector.affine_mul_reduce(
                                out=p[:, hs], accum_out=ph[:], in0=rkb[:, hs], in1=Sh[:],
                                scale=rqwc[:, t:t + 1], bias=float(w0))
                            psums.append(ph)
                        # ln(ve + qn2) ; then e' = w2*sqrt(ve), accumulating
                        # sum(e') so sum(scores) = sum(p) - sum(e') is free
                        esum = cols.tile([128, 1], dt.float32, name="esum", tag="esum")
                        nc.scalar.activation(out=ve[:], in_=ve[:], func=AF.Ln,
                                             bias=qn2c[:, t:t + 1], scale=1.0)
                        nc.scalar.activation(out=ve[:], in_=ve[:], func=AF.Exp,
                                             bias=lnw2_sb[:, 0:1], scale=0.5,
                                             accum_out=esum[:])
                        live[t] = (ve, p, psums, esum)

                    if 1 <= t <= NT:
                        u = t - 1
                        ve, p, psums, esum = live.pop(u)
                        # negmean = (sum(e') - sum(p)) / LK  (all accums free)
                        s1 = cols.tile([128, 1], dt.float32, name="s1", tag="s1")
                        nc.vector.tensor_add(out=s1[:], in0=psums[0][:], in1=psums[1][:])
                        negmean = cols.tile([128, 1], dt.float32, name="negmean", tag="negmean")
                        nc.vector.tensor_scalar(
                            out=negmean[:], in0=s1[:], scalar1=esum[:, 0:1],
                            scalar2=-1.0 / LK, op0=ALU.subtract, op1=ALU.mult)
                        # scores = p - e' in place; halves on DVE and gpsimd
                        nc.vector.scalar_tensor_tensor(
                            out=p[:, :LK // 2], in0=ve[:, :LK // 2], scalar=-1.0,
                            in1=p[:, :LK // 2], op0=ALU.mult, op1=ALU.add)
                        nc.gpsimd.tensor_sub(
                            out=p[:, LK // 2:], in0=p[:, LK // 2:],
                            in1=ve[:, LK // 2:])

                        # x = exp(scores - mean) (bf16), row sum
                        x = xpp.tile([128, LK], dt.bfloat16, name="x", tag="x")
                        xT = xtp.tile([128, LK // 128, 128], dt.bfloat16, name="xT", tag="xT")
                        rs = cols.tile([128, 1], dt.float32, name="rs", tag="rs")
                        if u == NT - 1:
                            # last tile: halve exp/transpose so the drain
                            # chain overlaps (transpose h0 runs during exp h1,
                            # and the ctx matmul's first 8 chunks start early)
                            sxa = cols.tile([128, 1], dt.float32, name="sxa", tag="sxa")
                            sxb = cols.tile([128, 1], dt.float32, name="sxb", tag="sxb")
                            nc.scalar.activation(out=x[:, :LK // 2], in_=p[:, :LK // 2],
                                                 func=AF.Exp, bias=negmean[:, 0:1],
                                                 scale=1.0, accum_out=sxa[:])
                            nc.sync.dma_start_transpose(xT[:, :LK // 256, :], x[:, :LK // 2])
                            nc.scalar.activation(out=x[:, LK // 2:], in_=p[:, LK // 2:],
                                                 func=AF.Exp, bias=negmean[:, 0:1],
                                                 scale=1.0, accum_out=sxb[:])
                            nc.sync.dma_start_transpose(xT[:, LK // 256:, :], x[:, LK // 2:])
                            sx = cols.tile([128, 1], dt.float32, name="sx", tag="sx")
                            nc.vector.tensor_add(out=sx[:], in0=sxa[:], in1=sxb[:])
                        else:
                            sx = cols.tile([128, 1], dt.float32, name="sx", tag="sx")
                            nc.scalar.activation(out=x[:], in_=p[:], func=AF.Exp,
                                                 bias=negmean[:, 0:1], scale=1.0,
                                                 accum_out=sx[:])
                            # xT[kk, c, i] = x[i, c*128+kk] via xbar transpose
                            nc.sync.dma_start_transpose(xT[:], x[:])
                        nc.vector.reciprocal(rs[:], sx[:])
                        live2[u] = (xT, rs)

                        # attn = x * (1/sum)  (gpsimd) -> DRAM
                        at = atp.tile([128, LK], dt.float32, name="at", tag="at")
                        nc.gpsimd.tensor_scalar(out=at[:], in0=x[:], scalar1=rs[:, 0:1],
                                                scalar2=None, op0=ALU.mult)
                        eng = nc.gpsimd if u == NT - 1 else nc.sync
                        eng.dma_start(attn_d[u * 128:(u + 1) * 128, :], at[:])

    nc.compile()
    return nc


def _get_graph(w0, w1, w2):
    key = (round(float(w0), 9), round(float(w1), 9), round(float(w2), 9))
    if key not in _GRAPH_CACHE:
        _GRAPH_CACHE[key] = _build_graph(*key)
    return _GRAPH_CACHE[key]


def kernel(query, key, value, Wq, bq, Wk, bk, scoring_weights):
    import ml_dtypes
    from concourse.bass_utils import run_bass_kernel_spmd

    query = np.asarray(query, dtype=np.float32)
    key_ = np.asarray(key, dtype=np.float32)
    value = np.asarray(value, dtype=np.float32)
    Wq = np.asarray(Wq, dtype=np.float32)
    bq = np.asarray(bq, dtype=np.float32)
    Wk = np.asarray(Wk, dtype=np.float32)
    bk = np.asarray(bk, dtype=np.float32)
    sw = np.asarray(scoring_weights, dtype=np.float64)

    e = np.exp(sw - sw.max())
    w = (e / e.sum()).astype(np.float64)
    w0, w1, w2 = float(w[0]), float(w[1]), float(w[2])

    nc = _get_graph(w0, w1, w2)

    wqT = np.ascontiguousarray(Wq.T)
    wkT = np.ascontiguousarray(Wk.T)
    bq_c = np.ascontiguousarray(bq.reshape(EC, 128).T)
    bk_c = np.ascontiguousarray(bk.reshape(EC, 128).T)
    ones = np.ones((128, 1), dtype=ml_dtypes.bfloat16)

    bf16 = ml_dtypes.bfloat16
    in_maps = []
    for c in range(NCORES):
        b, h = c // 2, c % 2
        in_maps.append({
            "qT": np.ascontiguousarray(query[b, h * QSH:(h + 1) * QSH, :].T),
            "kT": np.ascontiguousarray(key_[b].T),
            "v": np.ascontiguousarray(value[b]).astype(bf16),
            "wqT": wqT, "wkT": wkT, "bq": bq_c, "bk": bk_c, "ones": ones,
        })

    res = run_bass_kernel_spmd(nc, in_maps, core_ids=list(range(NCORES)))

    context = np.empty((B, LQ, D), dtype=np.float32)
    attn = np.empty((B, LQ, LK), dtype=np.float32)
    for c in range(NCORES):
        b, h = c // 2, c % 2
        context[b, h * QSH:(h + 1) * QSH] = res.results[c]["ctx"]
        attn[b, h * QSH:(h + 1) * QSH] = res.results[c]["attn"]

    return context, attn


# revision 70
# speedup vs baseline: 1.1927x; 1.0030x over previous
"""AdaptiveAttention Trainium2 kernel.

reference:
  q = tanh(query @ Wq.T + bq); k = tanh(key @ Wk.T + bk)
  dot = q @ k.T ; qn,kn row norms
  cos = dot / max(qn*kn, eps)
  euc = -sqrt(max(qn^2+kn^2-2dot, 0))
  w = softmax(scoring_weights); scores = w0*dot + w1*cos + w2*euc
  attn = softmax(scores, -1); context = attn @ v
  returns (context, attn)

Sharding: 8 cores = (batch b = c//2) x (query half h = c%2). Fully
data-parallel, no collectives. Each core: q rows [h*1024,(h+1)*1024) of
batch b, full K/V of batch b.

Device pipeline per core:
  phase 1: bf16 tanh transforms (tanh output stored as float32r for
           full-rate tf32-like matmuls), row norms via squares (gpsimd)
           + ones-matmul, rsqrt via Ln/Exp, DRAM-bounce row<->column
           reshapes and partition broadcasts.
  phase 2 (software-pipelined across query tiles, 3 stages): per
           128-query tile: S=q@k^T into PSUM (two halves so the next
           tile's matmuls start early), v=qn2+kn2-2S fused as
           scalar_tensor_tensor + qn2 via the Ln bias, e'=w2*sqrt(v)
           via Ln/Exp (one activation-table set), p=S*(w0+w1*rq*rk)
           in one affine_mul_reduce, scores=p-e' with the row-sum
           accumulated in the same op (softmax shifts by the row MEAN
           - spread is ~38 < 88 so exp can't overflow, and any shift
           cancels), x=exp(scores-mean) in bf16 with the row sum from
           the activation's accum_out, attn=x/sum on gpsimd, xT via
           the DMA xbar transpose, ctx=xT.T@v (bf16), row-scaled by
           1/sum on the scalar engine.
  Activation tables are pinned (exp/ln only in natural_log_exp_and_others,
  tanh only in tanh_and_derivative) to avoid table-load thrash.
  gpsimd (SWDGE) queues are used only for output DMAs - input DMAs on
  that path have broken completion ordering vs compute on HW.
"""

import sys

if "/opt/trn_rl_repo" not in sys.path:
    sys.path.insert(0, "/opt/trn_rl_repo")

import numpy as np

B, LQ, LK, D = 4, 2048, 2048, 512
NCORES = 8
QSH = LQ // 2          # query rows per core
NT = QSH // 128        # 8 query tiles per core
KB = LK // 512         # 4 k blocks of 512
DC = D // 128          # 4 contraction chunks
EC = D // 128          # 4 output-feature chunks

_GRAPH_CACHE = {}
_TABLES_PATCHED = False


def _patch_act_tables():
    """Restrict exp/ln to natural_log_exp_and_others and tanh to
    tanh_and_derivative so the compiler's table-load pass can't alternate
    between sets (each ACT_TABLE_LOAD costs ~2.7us). Set ids must stay
    stable, so we keep the dict keys/order and only prune membership."""
    global _TABLES_PATCHED
    if _TABLES_PATCHED:
        return
    import concourse.bacc as bacc_mod

    orig = bacc_mod.get_activation_tables

    def patched(arch):
        out = {}
        for name, funcs in orig(arch).items():
            fs = set(funcs)
            if name != "natural_log_exp_and_others":
                fs = {f for f in fs if f.name not in ("Exp", "Ln")}
            if name != "tanh_and_derivative":
                fs = {f for f in fs if f.name != "Tanh"}
            out[name] = fs
        return out

    bacc_mod.get_activation_tables = patched
    _TABLES_PATCHED = True


def _build_graph(w0, w1, w2):
    import concourse.bass as bass
    import concourse.tile as tile
    from concourse import bacc, mybir
    from concourse.tile_rust import add_dep_helper

    _patch_act_tables()

    dt = mybir.dt
    AF = mybir.ActivationFunctionType
    ALU = mybir.AluOpType

    lnw2 = float(np.log(w2))

    nc = bacc.Bacc(None, target_bir_lowering=False)

    qT_d = nc.dram_tensor("qT", [D, QSH], dt.float32r, kind="ExternalInput")
    kT_d = nc.dram_tensor("kT", [D, LK], dt.float32r, kind="ExternalInput")
    v_d = nc.dram_tensor("v", [LK, D], dt.bfloat16, kind="ExternalInput")
    wqT_d = nc.dram_tensor("wqT", [D, D], dt.float32r, kind="ExternalInput")
    wkT_d = nc.dram_tensor("wkT", [D, D], dt.float32r, kind="ExternalInput")
    bq_d = nc.dram_tensor("bq", [128, EC], dt.float32, kind="ExternalInput")
    bk_d = nc.dram_tensor("bk", [128, EC], dt.float32, kind="ExternalInput")
    ones_d = nc.dram_tensor("ones", [128, 1], dt.bfloat16, kind="ExternalInput")
    ctx_d = nc.dram_tensor("ctx", [QSH, D], dt.float32, kind="ExternalOutput")
    attn_d = nc.dram_tensor("attn", [QSH, LK], dt.float32, kind="ExternalOutput")

    with tile.TileContext(nc) as tc:
        with tc.tile_pool(name="static", bufs=1) as st, \
             tc.tile_pool(name="strow", bufs=1) as strow, \
             tc.tile_pool(name="dram", bufs=1, space="DRAM") as drp:
            # persistent SBUF
            kTt = [st.tile([128, LK], dt.float32r, name=f"kTt{i}", tag=f"kTt{i}") for i in range(EC)]
            qTt = [st.tile([128, QSH], dt.float32r, name=f"qTt{i}", tag=f"qTt{i}") for i in range(EC)]
            vbt = st.tile([128, LK // 128, D], dt.bfloat16, name="vbt", tag="vbt")
            kn2b = st.tile([128, LK], dt.float32, name="kn2b", tag="kn2b")
            rkb = st.tile([128, LK], dt.float32, name="rkb", tag="rkb")
            qn2c = strow.tile([128, NT], dt.float32, name="qn2c", tag="qn2c")
            rqwc = strow.tile([128, NT], dt.float32, name="rqwc", tag="rqwc")
            ones_sb = strow.tile([128, 1], dt.bfloat16, name="ones", tag="ones")
            bq_sb = strow.tile([128, EC], dt.float32, name="bqs", tag="bqs")
            bk_sb = strow.tile([128, EC], dt.float32, name="bks", tag="bks")
            lnw2_sb = strow.tile([128, 1], dt.float32, name="lnw2c", tag="lnw2c")
            nc.vector.memset(lnw2_sb[:], lnw2)
            nc.sync.dma_start(ones_sb[:], ones_d[:])
            nc.sync.dma_start(bq_sb[:], bq_d[:])
            nc.sync.dma_start(bk_sb[:], bk_d[:])

            # DRAM scratch for row<->col moves
            kn2_dram = drp.tile([1, LK], dt.float32, name="kn2d")
            qn2_dram = drp.tile([1, QSH], dt.float32, name="qn2d")

            # ---------------- phase 1: transforms + norms ----------------
            with tc.tile_pool(name="raw", bufs=1) as raw, \
                 tc.tile_pool(name="sq", bufs=1) as sqp, \
                 tc.tile_pool(name="rows", bufs=2) as rows, \
                 tc.tile_pool(name="p1ps", bufs=4, space="PSUM") as p1ps:
                kraw = [raw.tile([128, LK], dt.float32r, name=f"kraw{i}", tag=f"kraw{i}") for i in range(DC)]
                qraw = [raw.tile([128, QSH], dt.float32r, name=f"qraw{i}", tag=f"qraw{i}") for i in range(DC)]
                wq_sb = [raw.tile([128, D], dt.float32r, name=f"wq{i}", tag=f"wq{i}") for i in range(DC)]
                wk_sb = [raw.tile([128, D], dt.float32r, name=f"wk{i}", tag=f"wk{i}") for i in range(DC)]
                # weights first, then k/q raw column-sliced so transform work
                # unblocks after the first slice of each chunk lands
                for i in range(DC):
                    nc.sync.dma_start(wk_sb[i][:], wkT_d[i * 128:(i + 1) * 128, :])
                    nc.scalar.dma_start(wq_sb[i][:], wqT_d[i * 128:(i + 1) * 128, :])
                for js in range(KB):
                    for i in range(DC):
                        nc.sync.dma_start(kraw[i][:, js * 512:(js + 1) * 512],
                                          kT_d[i * 128:(i + 1) * 128, js * 512:(js + 1) * 512])
                for js in range(QSH // 512):
                    for i in range(DC):
                        nc.scalar.dma_start(qraw[i][:, js * 512:(js + 1) * 512],
                                          qT_d[i * 128:(i + 1) * 128, js * 512:(js + 1) * 512])

                # k transform: kTt[E][:, js] = tanh(sum_dc wkT[dc][:,E*128:] ^T @ kraw[dc][:, js] + bk[E])
                ksq = [sqp.tile([128, LK], dt.bfloat16, name=f"ksq{i}", tag=f"ksq{i}") for i in range(EC)]
                qsq = [sqp.tile([128, QSH], dt.bfloat16, name=f"qsq{i}", tag=f"qsq{i}") for i in range(EC)]
                kn2row = rows.tile([1, LK], dt.float32, name="kn2row", tag="kn2row")
                qn2row = rows.tile([1, QSH], dt.float32, name="qn2row", tag="qn2row")
                for js in range(KB):
                    sl = slice(js * 512, (js + 1) * 512)
                    for E in range(EC):
                        ps = p1ps.tile([128, 512], dt.float32, name="tps", tag="tps")
                        for dc in range(DC):
                            nc.tensor.matmul(
                                ps[:], wk_sb[dc][:, E * 128:(E + 1) * 128],
                                kraw[dc][:, sl],
                                start=(dc == 0), stop=(dc == DC - 1))
                        nc.scalar.activation(
                            out=kTt[E][:, sl], in_=ps[:],
                            func=AF.Tanh, bias=bk_sb[:, E:E + 1], scale=1.0)
                        nc.gpsimd.tensor_tensor(
                            out=ksq[E][:, sl], in0=kTt[E][:, sl].bitcast(dt.float32),
                            in1=kTt[E][:, sl].bitcast(dt.float32), op=ALU.mult)
                    # norm slice for this js as soon as its squares exist
                    nps = p1ps.tile([1, 512], dt.float32, name="nps", tag="tps")
                    for E in range(EC):
                        nc.tensor.matmul(nps[:], ones_sb[:], ksq[E][:, sl],
                                         start=(E == 0), stop=(E == EC - 1))
                    nc.vector.tensor_copy(out=kn2row[:, sl], in_=nps[:])

                # kn2 broadcast + column form (no ACT needed -> runs during q side)
                nc.scalar.dma_start(kn2_dram[:], kn2row[:])
                bc_src = bass.AP(tensor=kn2_dram.tensor, offset=kn2_dram[:].offset,
                                 ap=[[0, 128], [1, LK]])
                nc.sync.dma_start(kn2b[:], bc_src)
                kn2col = rows.tile([128, LK // 128], dt.float32, name="kn2col", tag="kn2col")
                kcol_src = bass.AP(tensor=kn2_dram.tensor, offset=kn2_dram[:].offset,
                                   ap=[[1, 128], [128, LK // 128]])
                nc.scalar.dma_start(kn2col[:], kcol_src)

                # q transform (+ squares + norm slices)
                for js in range(QSH // 512):
                    sl = slice(js * 512, (js + 1) * 512)
                    for E in range(EC):
                        ps = p1ps.tile([128, 512], dt.float32, name="tps", tag="tps")
                        for dc in range(DC):
                            nc.tensor.matmul(
                                ps[:], wq_sb[dc][:, E * 128:(E + 1) * 128],
                                qraw[dc][:, sl],
                                start=(dc == 0), stop=(dc == DC - 1))
                        last_tanh = nc.scalar.activation(
                            out=qTt[E][:, sl], in_=ps[:],
                            func=AF.Tanh, bias=bq_sb[:, E:E + 1], scale=1.0)
                        nc.gpsimd.tensor_tensor(
                            out=qsq[E][:, sl], in0=qTt[E][:, sl].bitcast(dt.float32),
                            in1=qTt[E][:, sl].bitcast(dt.float32), op=ALU.mult)
                    nps = p1ps.tile([1, 512], dt.float32, name="nps", tag="tps")
                    for E in range(EC):
                        nc.tensor.matmul(nps[:], ones_sb[:], qsq[E][:, sl],
                                         start=(E == 0), stop=(E == EC - 1))
                    nc.vector.tensor_copy(out=qn2row[:, sl], in_=nps[:])

                # qn2 row -> per-tile columns [128, NT] via DRAM bounce
                nc.scalar.dma_start(qn2_dram[:], qn2row[:])
                col_src = bass.AP(tensor=qn2_dram.tensor, offset=qn2_dram[:].offset,
                                  ap=[[1, 128], [128, NT]])
                nc.scalar.dma_start(qn2c[:], col_src)

                # rsqrts (single table switch to the ln/exp set; the dep
                # edge keeps them after ALL tanh ops so the scheduler cannot
                # interleave them and thrash the activation tables)
                rkcol = rows.tile([128, LK // 128], dt.float32, name="rkcol", tag="rkcol")
                first_ln = nc.scalar.activation(out=rkcol[:], in_=kn2col[:], func=AF.Ln)
                add_dep_helper(first_ln.ins, last_tanh.ins,
                               reason="rsqrt after all tanh (act-table order)")
                nc.scalar.activation(out=rkcol[:], in_=rkcol[:], func=AF.Exp, scale=-0.5)
                rk_dram = drp.tile([1, LK], dt.float32, name="rkd")
                rkd_dst = bass.AP(tensor=rk_dram.tensor, offset=rk_dram[:].offset,
                                  ap=[[1, 128], [128, LK // 128]])
                nc.scalar.dma_start(rkd_dst, rkcol[:])
                rkb_src = bass.AP(tensor=rk_dram.tensor, offset=rk_dram[:].offset,
                                  ap=[[0, 128], [1, LK]])
                nc.sync.dma_start(rkb[:], rkb_src)
                # rqw = w1 * rsqrt(qn2)
                nc.scalar.activation(out=rqwc[:], in_=qn2c[:], func=AF.Ln)
                nc.scalar.activation(out=rqwc[:], in_=rqwc[:], func=AF.Exp, scale=-0.5)
                nc.vector.tensor_scalar_mul(rqwc[:], rqwc[:], float(w1))

                # one 2MB v load: vbt[p, j, :] = v[j*128+p, :]
                v_src = bass.AP(tensor=v_d, offset=0,
                                ap=[[D, 128], [128 * D, LK // 128], [1, D]])
                nc.sync.dma_start(vbt[:], v_src)

            # ---------------- phase 2: attention ----------------
            with tc.tile_pool(name="sps", bufs=3, space="PSUM") as sps, \
                 tc.tile_pool(name="cps", bufs=2, space="PSUM") as cpsp, \
                 tc.tile_pool(name="ve", bufs=3) as vep, \
                 tc.tile_pool(name="pp", bufs=3) as ppp, \
                 tc.tile_pool(name="xp", bufs=3) as xpp, \
                 tc.tile_pool(name="xt", bufs=3) as xtp, \
                 tc.tile_pool(name="at", bufs=3) as atp, \
                 tc.tile_pool(name="cs", bufs=2) as csp, \
                 tc.tile_pool(name="cols", bufs=16) as cols:
                # software-pipelined by one tile: iteration t emits the head
                # of tile t (S matmuls + the two PSUM readers + ln/e') and the
                # tail of tile t-1 (scores, exp, attn, transpose, ctx) so the
                # serial v->ln->e'->scores->exp chain overlaps across tiles.
                live = {}
                live2 = {}
                for t in range(NT + 2):
                    if t >= 2:
                        u2 = t - 2
                        xT, rs = live2.pop(u2)
                        # ctx = x @ v (unnormalized), then scale rows by 1/sum
                        cps = cpsp.tile([128, D], dt.float32, name="cps", tag="cps")
                        for kc in range(LK // 128):
                            nc.tensor.matmul(cps[:], xT[:, kc, :], vbt[:, kc, :],
                                             start=(kc == 0), stop=(kc == LK // 128 - 1))
                        csb = csp.tile([128, D], dt.float32, name="csb", tag="csb")
                        nc.scalar.activation(out=csb[:, :D // 2], in_=cps[:, :D // 2],
                                             func=AF.Copy, scale=rs[:, 0:1])
                        nc.vector.tensor_scalar_mul(csb[:, D // 2:], cps[:, D // 2:],
                                                    rs[:, 0:1])
                        ceng = nc.gpsimd if u2 >= NT - 2 else nc.sync
                        ceng.dma_start(ctx_d[u2 * 128:(u2 + 1) * 128, :], csb[:])

                    if t < NT:
                        # S in two psum halves so the next tile's matmuls can
                        # start as soon as the first half's readers are done
                        ve = vep.tile([128, LK], dt.float32, name="ve", tag="ve")
                        p = ppp.tile([128, LK], dt.float32, name="p", tag="p")
                        psums = []
                        for h in range(2):
                            hs = slice(h * 1024, (h + 1) * 1024)
                            Sh = sps.tile([128, LK // 2], dt.float32, name=f"S{h}", tag="S")
                            for kb in range(2):
                                for dc in range(DC):
                                    nc.tensor.matmul(
                                        Sh[:, kb * 512:(kb + 1) * 512],
                                        qTt[dc][:, t * 128:(t + 1) * 128],
                                        kTt[dc][:, (2 * h + kb) * 512:(2 * h + kb + 1) * 512],
                                        start=(dc == 0), stop=(dc == DC - 1))
                            # ve = kn2 - 2S
                            nc.vector.scalar_tensor_tensor(
                                out=ve[:, hs], in0=Sh[:], scalar=-2.0, in1=kn2b[:, hs],
                                op0=ALU.mult, op1=ALU.add)
                            # p = (rk*w1*rq + w0) * S  (frees this S half);
                            # accum gives sum(p) for the softmax mean shift
                            ph = cols.tile([128, 1], dt.float32, name=f"ps{h}", tag=f"ps{h}")
                            nc.vector.affine_mul_reduce(
                                out=p[:, hs], accum_out=ph[:], in0=rkb[:, hs], in1=Sh[:],
                                scale=rqwc[:, t:t + 1], bias=float(w0))
                            psums.append(ph)
                        # ln(ve + qn2) ; then e' = w2*sqrt(ve), accumulating
                        # sum(e') so sum(scores) = sum(p) - sum(e') is free
                        esum = cols.tile([128, 1], dt.float32, name="esum", tag="esum")
                        nc.scalar.activation(out=ve[:], in_=ve[:], func=AF.Ln,
                                             bias=qn2c[:, t:t + 1], scale=1.0)
                        nc.scalar.activation(out=ve[:], in_=ve[:], func=AF.Exp,
                                             bias=lnw2_sb[:, 0:1], scale=0.5,
                                             accum_out=esum[:])
                        live[t] = (ve, p, psums, esum)

                    if 1 <= t <= NT:
                        u = t - 1
                        ve, p, psums, esum = live.pop(u)
                        # negmean = (sum(e') - sum(p)) / LK  (all accums free)
                        s1 = cols.tile([128, 1], dt.float32, name="s1", tag="s1")
                        nc.vector.tensor_add(out=s1[:], in0=psums[0][:], in1=psums[1][:])
                        negmean = cols.tile([128, 1], dt.float32, name="negmean", tag="negmean")
                        nc.vector.tensor_scalar(
                            out=negmean[:], in0=s1[:], scalar1=esum[:, 0:1],
                            scalar2=-1.0 / LK, op0=ALU.subtract, op1=ALU.mult)
                        # scores = p - e' in place; halves on DVE and gpsimd
                        nc.vector.scalar_tensor_tensor(
                            out=p[:, :LK // 2], in0=ve[:, :LK // 2], scalar=-1.0,
                            in1=p[:, :LK // 2], op0=ALU.mult, op1=ALU.add)
                        nc.gpsimd.tensor_sub(
                            out=p[:, LK // 2:], in0=p[:, LK // 2:],
                            in1=ve[:, LK // 2:])

                        # x = exp(scores - mean) (bf16), row sum
                        x = xpp.tile([128, LK], dt.bfloat16, name="x", tag="x")
                        xT = xtp.tile([128, LK // 128, 128], dt.bfloat16, name="xT", tag="xT")
                        rs = cols.tile([128, 1], dt.float32, name="rs", tag="rs")
                        if u == NT - 1:
                            # last tile: halve exp/transpose so the drain
                            # chain overlaps (transpose h0 runs during exp h1,
                            # and the ctx matmul's first 8 chunks start early)
                            sxa = cols.tile([128, 1], dt.float32, name="sxa", tag="sxa")
                            sxb = cols.tile([128, 1], dt.float32, name="sxb", tag="sxb")
                            nc.scalar.activation(out=x[:, :LK // 2], in_=p[:, :LK // 2],
                                                 func=AF.Exp, bias=negmean[:, 0:1],
                                                 scale=1.0, accum_out=sxa[:])
                            nc.sync.dma_start_transpose(xT[:, :LK // 256, :], x[:, :LK // 2])
                            nc.scalar.activation(out=x[:, LK // 2:], in_=p[:, LK // 2:],
                                                 func=AF.Exp, bias=negmean[:, 0:1],
                                                 scale=1.0, accum_out=sxb[:])
                            nc.sync.dma_start_transpose(xT[:, LK // 256:, :], x[:, LK // 2:])
                            sx = cols.tile([128, 1], dt.float32, name="sx", tag="sx")
                            nc.vector.tensor_add(out=sx[:], in0=sxa[:], in1=sxb[:])
                        else:
                            sx = cols.tile([128, 1], dt.float32, name="sx", tag="sx")
                            nc.scalar.activation(out=x[:], in_=p[:], func=AF.Exp,
                                                 bias=negmean[:, 0:1], scale=1.0,
                                                 accum_out=sx[:])
                            # xT[kk, c, i] = x[i, c*128+kk] via xbar transpose
                            nc.sync.dma_start_transpose(xT[:], x[:])
                        nc.vector.reciprocal(rs[:], sx[:])
                        live2[u] = (xT, rs)

                        # attn = x * (1/sum) -> DRAM
                        at = atp.tile([128, LK], dt.float32, name="at", tag="at")
                        if u == NT - 1:
                            # drain tail: halves on two engines / two queues
                            nc.gpsimd.tensor_scalar(out=at[:, :LK // 2], in0=x[:, :LK // 2],
                                                    scalar1=rs[:, 0:1], scalar2=None,
                                                    op0=ALU.mult)
                            nc.sync.dma_start(attn_d[u * 128:(u + 1) * 128, :LK // 2],
                                              at[:, :LK // 2])
                            nc.vector.tensor_scalar_mul(at[:, LK // 2:], x[:, LK // 2:],
                                                        rs[:, 0:1])
                            nc.gpsimd.dma_start(attn_d[u * 128:(u + 1) * 128, LK // 2:],
                                                at[:, LK // 2:])
                        else:
                            nc.gpsimd.tensor_scalar(out=at[:], in0=x[:], scalar1=rs[:, 0:1],
                                                    scalar2=None, op0=ALU.mult)
                            nc.sync.dma_start(attn_d[u * 128:(u + 1) * 128, :], at[:])

    nc.compile()
    return nc


def _get_graph(w0, w1, w2):
    key = (round(float(w0), 9), round(float(w1), 9), round(float(w2), 9))
    if key not in _GRAPH_CACHE:
        _GRAPH_CACHE[key] = _build_graph(*key)
    return _GRAPH_CACHE[key]


def kernel(query, key, value, Wq, bq, Wk, bk, scoring_weights):
    import ml_dtypes
    from concourse.bass_utils import run_bass_kernel_spmd

    query = np.asarray(query, dtype=np.float32)
    key_ = np.asarray(key, dtype=np.float32)
    value = np.asarray(value, dtype=np.float32)
    Wq = np.asarray(Wq, dtype=np.float32)
    bq = np.asarray(bq, dtype=np.float32)
    Wk = np.asarray(Wk, dtype=np.float32)
    bk = np.asarray(bk, dtype=np.float32)
    sw = np.asarray(scoring_weights, dtype=np.float64)

    e = np.exp(sw - sw.max())
    w = (e / e.sum()).astype(np.float64)
    w0, w1, w2 = float(w[0]), float(w[1]), float(w[2])

    nc = _get_graph(w0, w1, w2)

    wqT = np.ascontiguousarray(Wq.T)
    wkT = np.ascontiguousarray(Wk.T)
    bq_c = np.ascontiguousarray(bq.reshape(EC, 128).T)
    bk_c = np.ascontiguousarray(bk.reshape(EC, 128).T)
    ones = np.ones((128, 1), dtype=ml_dtypes.bfloat16)

    bf16 = ml_dtypes.bfloat16
    in_maps = []
    for c in range(NCORES):
        b, h = c // 2, c % 2
        in_maps.append({
            "qT": np.ascontiguousarray(query[b, h * QSH:(h + 1) * QSH, :].T),
            "kT": np.ascontiguousarray(key_[b].T),
            "v": np.ascontiguousarray(value[b]).astype(bf16),
            "wqT": wqT, "wkT": wkT, "bq": bq_c, "bk": bk_c, "ones": ones,
        })

    res = run_bass_kernel_spmd(nc, in_maps, core_ids=list(range(NCORES)))

    context = np.empty((B, LQ, D), dtype=np.float32)
    attn = np.empty((B, LQ, LK), dtype=np.float32)
    for c in range(NCORES):
        b, h = c // 2, c % 2
        context[b, h * QSH:(h + 1) * QSH] = res.results[c]["ctx"]
        attn[b, h * QSH:(h + 1) * QSH] = res.results[c]["attn"]

    return context, attn


# revision 81
# speedup vs baseline: 1.1974x; 1.0040x over previous
"""AdaptiveAttention Trainium2 kernel.

reference:
  q = tanh(query @ Wq.T + bq); k = tanh(key @ Wk.T + bk)
  dot = q @ k.T ; qn,kn row norms
  cos = dot / max(qn*kn, eps)
  euc = -sqrt(max(qn^2+kn^2-2dot, 0))
  w = softmax(scoring_weights); scores = w0*dot + w1*cos + w2*euc
  attn = softmax(scores, -1); context = attn @ v
  returns (context, attn)

Sharding: 8 cores = (batch b = c//2) x (query half h = c%2). Fully
data-parallel, no collectives. Each core: q rows [h*1024,(h+1)*1024) of
batch b, full K/V of batch b.

Device pipeline per core:
  phase 1: bf16 tanh transforms (tanh output stored as float32r for
           full-rate tf32-like matmuls), row norms via squares (gpsimd)
           + ones-matmul, rsqrt via Ln/Exp, DRAM-bounce row<->column
           reshapes and partition broadcasts.
  phase 2 (software-pipelined across query tiles, 3 stages): per
           128-query tile: S=q@k^T into PSUM (two halves so the next
           tile's matmuls start early), v=qn2+kn2-2S fused as
           scalar_tensor_tensor + qn2 via the Ln bias, e'=w2*sqrt(v)
           via Ln/Exp (one activation-table set), p=S*(w0+w1*rq*rk)
           in one affine_mul_reduce, scores=p-e' with the row-sum
           accumulated in the same op (softmax shifts by the row MEAN
           - spread is ~38 < 88 so exp can't overflow, and any shift
           cancels), x=exp(scores-mean) in bf16 with the row sum from
           the activation's accum_out, attn=x/sum on gpsimd, xT via
           the DMA xbar transpose, ctx=xT.T@v (bf16), row-scaled by
           1/sum on the scalar engine.
  Activation tables are pinned (exp/ln only in natural_log_exp_and_others,
  tanh only in tanh_and_derivative) to avoid table-load thrash.
  gpsimd (SWDGE) queues are used only for output DMAs - input DMAs on
  that path have broken completion ordering vs compute on HW.
"""

import sys

if "/opt/trn_rl_repo" not in sys.path:
    sys.path.insert(0, "/opt/trn_rl_repo")

import numpy as np

B, LQ, LK, D = 4, 2048, 2048, 512
NCORES = 8
QSH = LQ // 2          # query rows per core
NT = QSH // 128        # 8 query tiles per core
KB = LK // 512         # 4 k blocks of 512
DC = D // 128          # 4 contraction chunks
EC = D // 128          # 4 output-feature chunks

_GRAPH_CACHE = {}
_TABLES_PATCHED = False


def _patch_act_tables():
    """Restrict exp/ln to natural_log_exp_and_others and tanh to
    tanh_and_derivative so the compiler's table-load pass can't alternate
    between sets (each ACT_TABLE_LOAD costs ~2.7us). Set ids must stay
    stable, so we keep the dict keys/order and only prune membership."""
    global _TABLES_PATCHED
    if _TABLES_PATCHED:
        return
    import concourse.bacc as bacc_mod

    orig = bacc_mod.get_activation_tables

    def patched(arch):
        out = {}
        for name, funcs in orig(arch).items():
            fs = set(funcs)
            if name != "natural_log_exp_and_others":
                fs = {f for f in fs if f.name not in ("Exp", "Ln")}
            if name != "tanh_and_derivative":
                fs = {f for f in fs if f.name != "Tanh"}
            out[name] = fs
        return out

    bacc_mod.get_activation_tables = patched
    _TABLES_PATCHED = True


def _build_graph(w0, w1, w2):
    import concourse.bass as bass
    import concourse.tile as tile
    from concourse import bacc, mybir
    from concourse.tile_rust import add_dep_helper

    _patch_act_tables()

    dt = mybir.dt
    AF = mybir.ActivationFunctionType
    ALU = mybir.AluOpType

    lnw2 = float(np.log(w2))

    nc = bacc.Bacc(None, target_bir_lowering=False)

    qT_d = nc.dram_tensor("qT", [D, QSH], dt.float32r, kind="ExternalInput")
    kT_d = nc.dram_tensor("kT", [D, LK], dt.float32r, kind="ExternalInput")
    v_d = nc.dram_tensor("v", [LK, D], dt.bfloat16, kind="ExternalInput")
    wqT_d = nc.dram_tensor("wqT", [D, D], dt.float32r, kind="ExternalInput")
    wkT_d = nc.dram_tensor("wkT", [D, D], dt.float32r, kind="ExternalInput")
    bq_d = nc.dram_tensor("bq", [128, EC], dt.float32, kind="ExternalInput")
    bk_d = nc.dram_tensor("bk", [128, EC], dt.float32, kind="ExternalInput")
    ones_d = nc.dram_tensor("ones", [128, 1], dt.bfloat16, kind="ExternalInput")
    ctx_d = nc.dram_tensor("ctx", [QSH, D], dt.float32, kind="ExternalOutput")
    attn_d = nc.dram_tensor("attn", [QSH, LK], dt.float32, kind="ExternalOutput")

    with tile.TileContext(nc) as tc:
        with tc.tile_pool(name="static", bufs=1) as st, \
             tc.tile_pool(name="strow", bufs=1) as strow, \
             tc.tile_pool(name="dram", bufs=1, space="DRAM") as drp:
            # persistent SBUF
            kTt = [st.tile([128, LK], dt.float32r, name=f"kTt{i}", tag=f"kTt{i}") for i in range(EC)]
            qTt = [st.tile([128, QSH], dt.float32r, name=f"qTt{i}", tag=f"qTt{i}") for i in range(EC)]
            vbt = st.tile([128, LK // 128, D], dt.bfloat16, name="vbt", tag="vbt")
            kn2b = st.tile([128, LK], dt.float32, name="kn2b", tag="kn2b")
            rkb = st.tile([128, LK], dt.float32, name="rkb", tag="rkb")
            qn2c = strow.tile([128, NT], dt.float32, name="qn2c", tag="qn2c")
            rqwc = strow.tile([128, NT], dt.float32, name="rqwc", tag="rqwc")
            ones_sb = strow.tile([128, 1], dt.bfloat16, name="ones", tag="ones")
            bq_sb = strow.tile([128, EC], dt.float32, name="bqs", tag="bqs")
            bk_sb = strow.tile([128, EC], dt.float32, name="bks", tag="bks")
            lnw2_sb = strow.tile([128, 1], dt.float32, name="lnw2c", tag="lnw2c")
            nc.vector.memset(lnw2_sb[:], lnw2)
            nc.sync.dma_start(ones_sb[:], ones_d[:])
            nc.sync.dma_start(bq_sb[:], bq_d[:])
            nc.sync.dma_start(bk_sb[:], bk_d[:])

            # DRAM scratch for row<->col moves
            kn2_dram = drp.tile([1, LK], dt.float32, name="kn2d")
            qn2_dram = drp.tile([1, QSH], dt.float32, name="qn2d")

            # ---------------- phase 1: transforms + norms ----------------
            with tc.tile_pool(name="raw", bufs=1) as raw, \
                 tc.tile_pool(name="sq", bufs=1) as sqp, \
                 tc.tile_pool(name="rows", bufs=2) as rows, \
                 tc.tile_pool(name="p1ps", bufs=4, space="PSUM") as p1ps, \
                 tc.tile_pool(name="npsp", bufs=2, space="PSUM") as npsp:
                kraw = [raw.tile([128, LK], dt.float32r, name=f"kraw{i}", tag=f"kraw{i}") for i in range(DC)]
                qraw = [raw.tile([128, QSH], dt.float32r, name=f"qraw{i}", tag=f"qraw{i}") for i in range(DC)]
                wq_sb = [raw.tile([128, D], dt.float32r, name=f"wq{i}", tag=f"wq{i}") for i in range(DC)]
                wk_sb = [raw.tile([128, D], dt.float32r, name=f"wk{i}", tag=f"wk{i}") for i in range(DC)]
                # weights first, then k/q raw column-sliced so transform work
                # unblocks after the first slice of each chunk lands
                for i in range(DC):
                    nc.sync.dma_start(wk_sb[i][:], wkT_d[i * 128:(i + 1) * 128, :])
                    nc.scalar.dma_start(wq_sb[i][:], wqT_d[i * 128:(i + 1) * 128, :])
                for js in range(KB):
                    for i in range(DC):
                        nc.sync.dma_start(kraw[i][:, js * 512:(js + 1) * 512],
                                          kT_d[i * 128:(i + 1) * 128, js * 512:(js + 1) * 512])
                for js in range(QSH // 512):
                    for i in range(DC):
                        nc.scalar.dma_start(qraw[i][:, js * 512:(js + 1) * 512],
                                          qT_d[i * 128:(i + 1) * 128, js * 512:(js + 1) * 512])

                # k transform: kTt[E][:, js] = tanh(sum_dc wkT[dc][:,E*128:] ^T @ kraw[dc][:, js] + bk[E])
                ksq = [sqp.tile([128, LK], dt.bfloat16, name=f"ksq{i}", tag=f"ksq{i}") for i in range(EC)]
                qsq = [sqp.tile([128, QSH], dt.bfloat16, name=f"qsq{i}", tag=f"qsq{i}") for i in range(EC)]
                kn2row = rows.tile([1, LK], dt.float32, name="kn2row", tag="kn2row")
                qn2row = rows.tile([1, QSH], dt.float32, name="qn2row", tag="qn2row")
                for js in range(KB):
                    sl = slice(js * 512, (js + 1) * 512)
                    for E in range(EC):
                        ps = p1ps.tile([128, 512], dt.float32, name="tps", tag="tps")
                        for dc in range(DC):
                            nc.tensor.matmul(
                                ps[:], wk_sb[dc][:, E * 128:(E + 1) * 128],
                                kraw[dc][:, sl],
                                start=(dc == 0), stop=(dc == DC - 1))
                        nc.scalar.activation(
                            out=kTt[E][:, sl], in_=ps[:],
                            func=AF.Tanh, bias=bk_sb[:, E:E + 1], scale=1.0)
                        nc.gpsimd.tensor_tensor(
                            out=ksq[E][:, sl], in0=kTt[E][:, sl].bitcast(dt.float32),
                            in1=kTt[E][:, sl].bitcast(dt.float32), op=ALU.mult)
                    # norm slice for this js as soon as its squares exist
                    nps = npsp.tile([1, 512], dt.float32, name="nps", tag="nps")
                    for E in range(EC):
                        nc.tensor.matmul(nps[:], ones_sb[:], ksq[E][:, sl],
                                         start=(E == 0), stop=(E == EC - 1))
                    nc.vector.tensor_copy(out=kn2row[:, sl], in_=nps[:])

                # kn2 broadcast + column form (no ACT needed -> runs during q side)
                nc.scalar.dma_start(kn2_dram[:], kn2row[:])
                kn2col = rows.tile([128, LK // 128], dt.float32, name="kn2col", tag="kn2col")
                kcol_src = bass.AP(tensor=kn2_dram.tensor, offset=kn2_dram[:].offset,
                                   ap=[[1, 128], [128, LK // 128]])
                nc.scalar.dma_start(kn2col[:], kcol_src)
                bc_src = bass.AP(tensor=kn2_dram.tensor, offset=kn2_dram[:].offset,
                                 ap=[[0, 128], [1, LK]])
                nc.sync.dma_start(kn2b[:], bc_src)

                # q transform (+ squares + norm slices)
                for js in range(QSH // 512):
                    sl = slice(js * 512, (js + 1) * 512)
                    for E in range(EC):
                        ps = p1ps.tile([128, 512], dt.float32, name="tps", tag="tps")
                        for dc in range(DC):
                            nc.tensor.matmul(
                                ps[:], wq_sb[dc][:, E * 128:(E + 1) * 128],
                                qraw[dc][:, sl],
                                start=(dc == 0), stop=(dc == DC - 1))
                        last_tanh = nc.scalar.activation(
                            out=qTt[E][:, sl], in_=ps[:],
                            func=AF.Tanh, bias=bq_sb[:, E:E + 1], scale=1.0)
                        nc.gpsimd.tensor_tensor(
                            out=qsq[E][:, sl], in0=qTt[E][:, sl].bitcast(dt.float32),
                            in1=qTt[E][:, sl].bitcast(dt.float32), op=ALU.mult)
                    nps = npsp.tile([1, 512], dt.float32, name="nps", tag="nps")
                    for E in range(EC):
                        nc.tensor.matmul(nps[:], ones_sb[:], qsq[E][:, sl],
                                         start=(E == 0), stop=(E == EC - 1))
                    nc.vector.tensor_copy(out=qn2row[:, sl], in_=nps[:])

                # qn2 row -> per-tile columns [128, NT] via DRAM bounce
                nc.scalar.dma_start(qn2_dram[:], qn2row[:])
                col_src = bass.AP(tensor=qn2_dram.tensor, offset=qn2_dram[:].offset,
                                  ap=[[1, 128], [128, NT]])
                nc.scalar.dma_start(qn2c[:], col_src)

                # rsqrts (single table switch; dep edge keeps them after all
                # tanh so the scheduler cannot thrash the activation tables)
                rkcol = rows.tile([128, LK // 128], dt.float32, name="rkcol", tag="rkcol")
                first_ln = nc.scalar.activation(out=rkcol[:], in_=kn2col[:], func=AF.Ln)
                add_dep_helper(first_ln.ins, last_tanh.ins,
                               reason="rsqrt after all tanh (act-table order)")
                nc.scalar.activation(out=rkcol[:], in_=rkcol[:], func=AF.Exp, scale=-0.5)
                rk_dram = drp.tile([1, LK], dt.float32, name="rkd")
                rkd_dst = bass.AP(tensor=rk_dram.tensor, offset=rk_dram[:].offset,
                                  ap=[[1, 128], [128, LK // 128]])
                nc.scalar.dma_start(rkd_dst, rkcol[:])
                rkb_src = bass.AP(tensor=rk_dram.tensor, offset=rk_dram[:].offset,
                                  ap=[[0, 128], [1, LK]])
                nc.sync.dma_start(rkb[:], rkb_src)
                # rqw = w1 * rsqrt(qn2)
                nc.scalar.activation(out=rqwc[:], in_=qn2c[:], func=AF.Ln)
                nc.scalar.activation(out=rqwc[:], in_=rqwc[:], func=AF.Exp, scale=-0.5)
                nc.vector.tensor_scalar_mul(rqwc[:], rqwc[:], float(w1))

                # one 2MB v load: vbt[p, j, :] = v[j*128+p, :]
                v_src = bass.AP(tensor=v_d, offset=0,
                                ap=[[D, 128], [128 * D, LK // 128], [1, D]])
                nc.sync.dma_start(vbt[:], v_src)

            # ---------------- phase 2: attention ----------------
            with tc.tile_pool(name="sps", bufs=3, space="PSUM") as sps, \
                 tc.tile_pool(name="cps", bufs=2, space="PSUM") as cpsp, \
                 tc.tile_pool(name="ve", bufs=3) as vep, \
                 tc.tile_pool(name="pp", bufs=3) as ppp, \
                 tc.tile_pool(name="xp", bufs=3) as xpp, \
                 tc.tile_pool(name="xt", bufs=3) as xtp, \
                 tc.tile_pool(name="at", bufs=3) as atp, \
                 tc.tile_pool(name="cs", bufs=2) as csp, \
                 tc.tile_pool(name="cols", bufs=16) as cols:
                # software-pipelined by one tile: iteration t emits the head
                # of tile t (S matmuls + the two PSUM readers + ln/e') and the
                # tail of tile t-1 (scores, exp, attn, transpose, ctx) so the
                # serial v->ln->e'->scores->exp chain overlaps across tiles.
                live = {}
                live2 = {}
                for t in range(NT + 2):
                    if t >= 2:
                        u2 = t - 2
                        xT, rs = live2.pop(u2)
                        # ctx = x @ v (unnormalized), then scale rows by 1/sum
                        cps = cpsp.tile([128, D], dt.float32, name="cps", tag="cps")
                        for kc in range(LK // 128):
                            nc.tensor.matmul(cps[:], xT[:, kc, :], vbt[:, kc, :],
                                             start=(kc == 0), stop=(kc == LK // 128 - 1))
                        csb = csp.tile([128, D], dt.float32, name="csb", tag="csb")
                        nc.scalar.activation(out=csb[:, :D // 2], in_=cps[:, :D // 2],
                                             func=AF.Copy, scale=rs[:, 0:1])
                        nc.vector.tensor_scalar_mul(csb[:, D // 2:], cps[:, D // 2:],
                                                    rs[:, 0:1])
                        ceng = nc.gpsimd if u2 >= NT - 2 else nc.sync
                        ceng.dma_start(ctx_d[u2 * 128:(u2 + 1) * 128, :], csb[:])

                    if t < NT:
                        # S in two psum halves so the next tile's matmuls can
                        # start as soon as the first half's readers are done
                        ve = vep.tile([128, LK], dt.float32, name="ve", tag="ve")
                        p = ppp.tile([128, LK], dt.float32, name="p", tag="p")
                        psums = []
                        for h in range(2):
                            hs = slice(h * 1024, (h + 1) * 1024)
                            Sh = sps.tile([128, LK // 2], dt.float32, name=f"S{h}", tag="S")
                            for kb in range(2):
                                for dc in range(DC):
                                    nc.tensor.matmul(
                                        Sh[:, kb * 512:(kb + 1) * 512],
                                        qTt[dc][:, t * 128:(t + 1) * 128],
                                        kTt[dc][:, (2 * h + kb) * 512:(2 * h + kb + 1) * 512],
                                        start=(dc == 0), stop=(dc == DC - 1))
                            # ve = kn2 - 2S
                            nc.vector.scalar_tensor_tensor(
                                out=ve[:, hs], in0=Sh[:], scalar=-2.0, in1=kn2b[:, hs],
                                op0=ALU.mult, op1=ALU.add)
                            # p = (rk*w1*rq + w0) * S  (frees this S half);
                            # accum gives sum(p) for the softmax mean shift
                            ph = cols.tile([128, 1], dt.float32, name=f"ps{h}", tag=f"ps{h}")
                            nc.vector.affine_mul_reduce(
                                out=p[:, hs], accum_out=ph[:], in0=rkb[:, hs], in1=Sh[:],
                                scale=rqwc[:, t:t + 1], bias=float(w0))
                            psums.append(ph)
                        # ln(ve + qn2) ; then e' = w2*sqrt(ve), accumulating
                        # sum(e') so sum(scores) = sum(p) - sum(e') is free
                        esum = cols.tile([128, 1], dt.float32, name="esum", tag="esum")
                        nc.scalar.activation(out=ve[:], in_=ve[:], func=AF.Ln,
                                             bias=qn2c[:, t:t + 1], scale=1.0)
                        nc.scalar.activation(out=ve[:], in_=ve[:], func=AF.Exp,
                                             bias=lnw2_sb[:, 0:1], scale=0.5,
                                             accum_out=esum[:])
                        live[t] = (ve, p, psums, esum)

                    if 1 <= t <= NT:
                        u = t - 1
                        ve, p, psums, esum = live.pop(u)
                        # negmean = (sum(e') - sum(p)) / LK  (all accums free)
                        s1 = cols.tile([128, 1], dt.float32, name="s1", tag="s1")
                        nc.vector.tensor_add(out=s1[:], in0=psums[0][:], in1=psums[1][:])
                        negmean = cols.tile([128, 1], dt.float32, name="negmean", tag="negmean")
                        nc.vector.tensor_scalar(
                            out=negmean[:], in0=s1[:], scalar1=esum[:, 0:1],
                            scalar2=-1.0 / LK, op0=ALU.subtract, op1=ALU.mult)
                        # scores = p - e' in place; halves on DVE and gpsimd
                        nc.vector.scalar_tensor_tensor(
                            out=p[:, :LK // 2], in0=ve[:, :LK // 2], scalar=-1.0,
                            in1=p[:, :LK // 2], op0=ALU.mult, op1=ALU.add)
                        nc.gpsimd.tensor_sub(
                            out=p[:, LK // 2:], in0=p[:, LK // 2:],
                            in1=ve[:, LK // 2:])

                        # x = exp(scores - mean) (bf16), row sum
                        x = xpp.tile([128, LK], dt.bfloat16, name="x", tag="x")
                        xT = xtp.tile([128, LK // 128, 128], dt.bfloat16, name="xT", tag="xT")
                        rs = cols.tile([128, 1], dt.float32, name="rs", tag="rs")
                        if u == NT - 1:
                            # last tile: halve exp/transpose so the drain
                            # chain overlaps (transpose h0 runs during exp h1,
                            # and the ctx matmul's first 8 chunks start early)
                            sxa = cols.tile([128, 1], dt.float32, name="sxa", tag="sxa")
                            sxb = cols.tile([128, 1], dt.float32, name="sxb", tag="sxb")
                            nc.scalar.activation(out=x[:, :LK // 2], in_=p[:, :LK // 2],
                                                 func=AF.Exp, bias=negmean[:, 0:1],
                                                 scale=1.0, accum_out=sxa[:])
                            nc.sync.dma_start_transpose(xT[:, :LK // 256, :], x[:, :LK // 2])
                            nc.scalar.activation(out=x[:, LK // 2:], in_=p[:, LK // 2:],
                                                 func=AF.Exp, bias=negmean[:, 0:1],
                                                 scale=1.0, accum_out=sxb[:])
                            nc.sync.dma_start_transpose(xT[:, LK // 256:, :], x[:, LK // 2:])
                            sx = cols.tile([128, 1], dt.float32, name="sx", tag="sx")
                            nc.vector.tensor_add(out=sx[:], in0=sxa[:], in1=sxb[:])
                        else:
                            sx = cols.tile([128, 1], dt.float32, name="sx", tag="sx")
                            nc.scalar.activation(out=x[:], in_=p[:], func=AF.Exp,
                                                 bias=negmean[:, 0:1], scale=1.0,
                                                 accum_out=sx[:])
                            # xT[kk, c, i] = x[i, c*128+kk] via xbar transpose
                            nc.sync.dma_start_transpose(xT[:], x[:])
                        nc.vector.reciprocal(rs[:], sx[:])
                        live2[u] = (xT, rs)

                        # attn = x * (1/sum) -> DRAM
                        at = atp.tile([128, LK], dt.float32, name="at", tag="at")
                        if u == NT - 1:
                            # drain tail: halves on two engines / two queues
                            nc.gpsimd.tensor_scalar(out=at[:, :LK // 2], in0=x[:, :LK // 2],
                                                    scalar1=rs[:, 0:1], scalar2=None,
                                                    op0=ALU.mult)
                            nc.sync.dma_start(attn_d[u * 128:(u + 1) * 128, :LK // 2],
                                              at[:, :LK // 2])
                            nc.vector.tensor_scalar_mul(at[:, LK // 2:], x[:, LK // 2:],
                                                        rs[:, 0:1])
                            nc.gpsimd.dma_start(attn_d[u * 128:(u + 1) * 128, LK // 2:],
                                                at[:, LK // 2:])
                        else:
                            nc.gpsimd.tensor_scalar(out=at[:], in0=x[:], scalar1=rs[:, 0:1],
                                                    scalar2=None, op0=ALU.mult)
                            nc.sync.dma_start(attn_d[u * 128:(u + 1) * 128, :], at[:])

    nc.compile()
    return nc


def _get_graph(w0, w1, w2):
    key = (round(float(w0), 9), round(float(w1), 9), round(float(w2), 9))
    if key not in _GRAPH_CACHE:
        _GRAPH_CACHE[key] = _build_graph(*key)
    return _GRAPH_CACHE[key]


def kernel(query, key, value, Wq, bq, Wk, bk, scoring_weights):
    import ml_dtypes
    from concourse.bass_utils import run_bass_kernel_spmd

    query = np.asarray(query, dtype=np.float32)
    key_ = np.asarray(key, dtype=np.float32)
    value = np.asarray(value, dtype=np.float32)
    Wq = np.asarray(Wq, dtype=np.float32)
    bq = np.asarray(bq, dtype=np.float32)
    Wk = np.asarray(Wk, dtype=np.float32)
    bk = np.asarray(bk, dtype=np.float32)
    sw = np.asarray(scoring_weights, dtype=np.float64)

    e = np.exp(sw - sw.max())
    w = (e / e.sum()).astype(np.float64)
    w0, w1, w2 = float(w[0]), float(w[1]), float(w[2])

    nc = _get_graph(w0, w1, w2)

    wqT = np.ascontiguousarray(Wq.T)
    wkT = np.ascontiguousarray(Wk.T)
    bq_c = np.ascontiguousarray(bq.reshape(EC, 128).T)
    bk_c = np.ascontiguousarray(bk.reshape(EC, 128).T)
    ones = np.ones((128, 1), dtype=ml_dtypes.bfloat16)

    bf16 = ml_dtypes.bfloat16
    in_maps = []
    for c in range(NCORES):
        b, h = c // 2, c % 2
        in_maps.append({
            "qT": np.ascontiguousarray(query[b, h * QSH:(h + 1) * QSH, :].T),
            "kT": np.ascontiguousarray(key_[b].T),
            "v": np.ascontiguousarray(value[b]).astype(bf16),
            "wqT": wqT, "wkT": wkT, "bq": bq_c, "bk": bk_c, "ones": ones,
        })

    res = run_bass_kernel_spmd(nc, in_maps, core_ids=list(range(NCORES)))

    context = np.empty((B, LQ, D), dtype=np.float32)
    attn = np.empty((B, LQ, LK), dtype=np.float32)
    for c in range(NCORES):
        b, h = c // 2, c % 2
        context[b, h * QSH:(h + 1) * QSH] = res.results[c]["ctx"]
        attn[b, h * QSH:(h + 1) * QSH] = res.results[c]["attn"]

    return context, attn
